# revision 9
# baseline (speedup 1.0000x reference)
"""Trainium2 Bass kernel for nn_DSnetwork (GNN message passing).

Computation (see reference):
    3x layers: h = elu(h @ W + b + (segmean(h) @ Ws + bs)[batch_idx])
    out = relu(segmean(h) @ Wf1 + bf1) @ Wf2 + bf2

batch_idx is sorted, so graphs are contiguous node ranges. Graphs are split
into 8 contiguous per-core ranges (node-balanced), and within a core into
chunks of <= C nodes covering whole graphs. Segment mean and the
gather-broadcast are matmuls against host-built 0/1 indicator matrices
(A: [node, graph] with recip folded in; AT: [graph, node] plus a ones row
that applies the per-layer bias directly in PSUM). We carry v = 1 + elu(x)
(= min(exp(x),1) + relu(x)) and fold the -1 into the next layer's bias via
column sums of W/Ws/Wf1.

Engine layout per chunk-layer:
  PE:   pool (8 acc MMs) + x2 + bias-row MM + xb (2x W + 2x gather MMs)
  DMA:  v -> hT transposes via the X-bar (dma_start_transpose), chunk loads
  ACT:  exp(xb), x2sb copy, head
  GPS:  emin = min(exp, 1)
  DVE:  pooledT copy, v = max(xb,0) + emin (STT straight from PSUM)
"""

import os
import sys

for _p in ("/opt/trn_rl_repo", "/root/.axon_site/_ro/trn_rl_repo"):
    if os.path.isdir(_p) and _p not in sys.path:
        sys.path.insert(0, _p)

from contextlib import ExitStack
from dataclasses import dataclass

import numpy as np

import concourse.bass as bass
import concourse.mybir as mybir
import concourse.tile as tile
from concourse import bacc, bass_utils

F16 = mybir.dt.float16
F32 = mybir.dt.float32
AF = mybir.ActivationFunctionType
OP = mybir.AluOpType


@dataclass(frozen=True)
class Cfg:
    N: int = 500000
    D: int = 128
    G: int = 10000
    T: int = 10
    n_cores: int = 8
    C: int = 1024  # nodes per chunk (multiple of 128)
    GCH: int = 64  # max graphs per chunk
    M: int = 4     # chunks per group (layer sweeps within a group)

    @property
    def BLK(self):
        return self.C // 128


CFG = Cfg()

# tuning knobs
POOLED_DVE = 1   # pooledT psum->sbuf copy on DVE (else ACT)
P3_DVE = 1       # head p3 copy on DVE (else ACT)
TRANS_SCALAR = 0  # issue hT dma transposes from ACT hwdge (else sync)
GPS_MIN = int(os.environ.get("GPS_MIN", "1"))      # emin on gpsimd (else DVE)
TRANS_DMA = int(os.environ.get("TRANS_DMA", "1"))  # hT via dma transpose (else PE)
VTRANS_DMA = int(os.environ.get("VTRANS_DMA", "1"))  # v via DRAM dma transpose (else PE)


# --------------------------------------------------------------------------
# Host-side preparation
# --------------------------------------------------------------------------

def _prepare(cfg, h, batch_idx, W, b, Ws, bs, Wf1, bf1, Wf2, bf2):
    """Pack graphs into per-core chunk arrays. Returns in_maps + assembly info."""
    N, D, G, T, C, GCH = cfg.N, cfg.D, cfg.G, cfg.T, cfg.C, cfg.GCH
    BLK = cfg.BLK
    bi = np.asarray(batch_idx).astype(np.int64)
    counts = np.bincount(bi, minlength=G)
    starts = np.zeros(G + 1, np.int64)
    np.cumsum(counts, out=starts[1:])
    assert counts.max() <= C, "single graph larger than a chunk"

    # split graphs into n_cores contiguous ranges with ~equal node counts
    targets = (np.arange(1, cfg.n_cores) * (N / cfg.n_cores)).astype(np.int64)
    cuts = np.searchsorted(starts[1 : G + 1], targets)
    core_g = np.concatenate([[0], cuts, [G]])

    # chunk packing per core
    core_chunks = []
    for c in range(cfg.n_cores):
        g0, g1 = int(core_g[c]), int(core_g[c + 1])
        chunks = []
        g = g0
        while g < g1:
            ge = g
            nodes = 0
            while ge < g1 and ge - g < GCH and nodes + counts[ge] <= C:
                nodes += int(counts[ge])
                ge += 1
            assert ge > g
            chunks.append((g, ge))
            g = ge
        core_chunks.append(chunks)
    nchunk = max(len(ch) for ch in core_chunks)

    # weights prep (shared across cores)
    W = [np.asarray(w, np.float32) for w in W]
    Ws = [np.asarray(w, np.float32) for w in Ws]
    b = [np.asarray(x, np.float32) for x in b]
    bs = [np.asarray(x, np.float32) for x in bs]
    Wf1 = np.asarray(Wf1, np.float32)
    bf1 = np.asarray(bf1, np.float32)
    Wf2 = np.asarray(Wf2, np.float32)
    bf2 = np.asarray(bf2, np.float32)

    brow = np.zeros((3, D), np.float32)
    for l in range(3):
        brow[l] = b[l] + bs[l]
        if l >= 1:  # inputs are v = h + 1 -> subtract column sums
            brow[l] -= W[l].sum(axis=0) + Ws[l].sum(axis=0)
    bf1_eff = bf1 - Wf1.sum(axis=0)  # pooled input is v = h + 1

    W_h = np.stack([w.astype(np.float16) for w in W])  # [3,128,128]
    Ws_h = np.stack([w.astype(np.float16) for w in Ws])
    bcol_h = brow.T.astype(np.float16).copy()  # [128, 3] f16, bias rows via MM
    bf1_col = bf1_eff.reshape(2, D).T.copy()  # [128, 2]
    Wf2_r = Wf2.reshape(2, D, T).copy()  # [2, 128, 10]
    bf2_col = bf2.reshape(T, 1).copy()

    h = np.ascontiguousarray(np.asarray(h, np.float32)).astype(np.float16)
    in_maps = []
    asm = []  # per core: (positions into [nchunk*GCH], graph ids)
    for c in range(cfg.n_cores):
        chunks = core_chunks[c]
        h_pad = np.zeros((nchunk * C, D), np.float16)       # node-major (for v transpose)
        hA = np.zeros((nchunk, 128, BLK * 128 + BLK * GCH), np.float16)
        AT = np.zeros((nchunk, GCH + 1, C), np.float16)
        AT[:, GCH, :] = 1.0  # ones row: applies bias row of x2sb
        pos_list = []
        gid_list = []
        for k, (gs, ge) in enumerate(chunks):
            n0, n1 = int(starts[gs]), int(starts[ge])
            nn = n1 - n0
            h_pad[k * C : k * C + nn] = h[n0:n1]
            # hT part: hA[k, p, b*128 + f] = h_pad[k*C + b*128 + p, f]
            hc = h_pad[k * C : (k + 1) * C].reshape(BLK, 128, D)
            hA[k, :, : BLK * 128] = hc.transpose(1, 0, 2).reshape(128, BLK * D)
            lidx = (bi[n0:n1] - gs).astype(np.int64)  # local graph idx per node
            narng = np.arange(nn)
            ng = ge - gs
            recip = np.zeros(GCH, np.float32)
            recip[:ng] = 1.0 / np.maximum(counts[gs:ge], 1)
            A = np.zeros((128, BLK, GCH), np.float16)
            A[narng % 128, narng // 128, lidx] = recip[lidx].astype(np.float16)
            hA[k, :, BLK * 128 :] = A.reshape(128, BLK * GCH)
            AT[k, lidx, narng] = 1.0
            pos_list.append(k * GCH + np.arange(ng))
            gid_list.append(np.arange(gs, ge))
        in_maps.append(
            {
                "hraw": h_pad,
                "hA": hA,
                "AT": AT,
                "W": W_h,
                "Wsm": Ws_h,
                "bcolh": bcol_h,
                "wf1": Wf1,
                "bf1c": bf1_col,
                "wf2": Wf2_r,
                "bf2c": bf2_col,
                "eye": np.eye(D, dtype=np.float16),
            }
        )
        asm.append(
            (
                np.concatenate(pos_list) if pos_list else np.zeros(0, np.int64),
                np.concatenate(gid_list) if gid_list else np.zeros(0, np.int64),
            )
        )

    empty_row = (
        np.maximum(bf1, 0.0) @ Wf2 + bf2 if (counts == 0).any() else None
    )
    return {
        "nchunk": nchunk,
        "in_maps": in_maps,
        "asm": asm,
        "counts": counts,
        "empty_row": empty_row,
    }


# --------------------------------------------------------------------------
# Device program
# --------------------------------------------------------------------------

def _build(cfg, nchunk, reps=1):
    D, T, C, GCH, BLK, M = cfg.D, cfg.T, cfg.C, cfg.GCH, cfg.BLK, cfg.M
    nc = bacc.Bacc("TRN2", target_bir_lowering=False, debug=False)

    hraw_d = nc.dram_tensor("hraw", [nchunk * C, D], F16, kind="ExternalInput").ap()
    hA_d = nc.dram_tensor("hA", [nchunk, 128, BLK * 128 + BLK * GCH], F16, kind="ExternalInput").ap()
    AT_d = nc.dram_tensor("AT", [nchunk, GCH + 1, C], F16, kind="ExternalInput").ap()
    W_d = nc.dram_tensor("W", [3, D, D], F16, kind="ExternalInput").ap()
    Ws_d = nc.dram_tensor("Wsm", [3, D, D], F16, kind="ExternalInput").ap()
    bcolh_d = nc.dram_tensor("bcolh", [D, 3], F16, kind="ExternalInput").ap()
    wf1_d = nc.dram_tensor("wf1", [D, 2 * D], F32, kind="ExternalInput").ap()
    bf1_d = nc.dram_tensor("bf1c", [D, 2], F32, kind="ExternalInput").ap()
    wf2_d = nc.dram_tensor("wf2", [2, D, T], F32, kind="ExternalInput").ap()
    bf2_d = nc.dram_tensor("bf2c", [T, 1], F32, kind="ExternalInput").ap()
    eye_d = nc.dram_tensor("eye", [D, D], F16, kind="ExternalInput").ap()
    out_d = nc.dram_tensor("out", [T, nchunk * GCH], F32, kind="ExternalOutput").ap()

    with tile.TileContext(nc) as tc, ExitStack() as ctx:
        const = ctx.enter_context(tc.tile_pool(name="const", bufs=1))
        gio = ctx.enter_context(tc.tile_pool(name="gio", bufs=M + 2))
        vp = ctx.enter_context(tc.tile_pool(name="vp", bufs=M + 2))
        hp = ctx.enter_context(tc.tile_pool(name="hp", bufs=3))
        sm = ctx.enter_context(tc.tile_pool(name="sm", bufs=3))
        ps_big = ctx.enter_context(tc.tile_pool(name="ps_big", bufs=2, space="PSUM"))
        ps_sm = ctx.enter_context(
            tc.tile_pool(name="ps_sm", bufs=2 if (TRANS_DMA and VTRANS_DMA) else 1, space="PSUM")
        )
        if not (TRANS_DMA and VTRANS_DMA):
            ps_tp = ctx.enter_context(tc.tile_pool(name="ps_tp", bufs=2, space="PSUM"))

        W_sb = const.tile([D, 3, D], F16, name="W_sb")
        nc.sync.dma_start(W_sb[:], W_d.rearrange("l k m -> k l m"))
        Ws_sb = const.tile([D, 3, D], F16, name="Ws_sb")
        nc.sync.dma_start(Ws_sb[:], Ws_d.rearrange("l k m -> k l m"))
        bcolh_sb = const.tile([D, 3], F16, name="bcolh_sb")
        nc.sync.dma_start(bcolh_sb[:], bcolh_d)
        wf1_sb = const.tile([D, 2 * D], F32, name="wf1_sb")
        nc.sync.dma_start(wf1_sb[:], wf1_d)
        bf1_sb = const.tile([D, 2], F32, name="bf1_sb")
        nc.sync.dma_start(bf1_sb[:], bf1_d)
        wf2_sb = const.tile([D, 2, T], F32, name="wf2_sb")
        nc.sync.dma_start(wf2_sb[:], wf2_d.rearrange("x k m -> k x m"))
        bf2_sb = const.tile([T, 1], F32, name="bf2_sb")
        nc.sync.dma_start(bf2_sb[:], bf2_d)
        eye_sb = const.tile([D, D], F16, name="eye_sb")
        nc.sync.dma_start(eye_sb[:], eye_d)
        out_sb = const.tile([T, nchunk * GCH], F32, name="out_sb")

        dma_tr = nc.scalar.dma_start_transpose if TRANS_SCALAR else nc.sync.dma_start_transpose

        def pe_transpose(dst, src_blk, on_act=True):
            """dst [128, BLK, 128] or [128, C] <- transpose of per-block APs."""
            tp_ps = ps_tp.tile([128, BLK, 128], F16, tag="tp")
            for bb in range(BLK):
                nc.tensor.transpose(tp_ps[:, bb, :], src_blk(bb), eye_sb[:])
            if on_act:
                nc.scalar.copy(dst[:], tp_ps[:] if dst.ndim == 3 else tp_ps[:].rearrange("p a b -> p (a b)"))
            else:
                nc.vector.tensor_copy(dst[:], tp_ps[:] if dst.ndim == 3 else tp_ps[:].rearrange("p a b -> p (a b)"))

        def pool_x2_xb(l, hT_blk, A_blk, AT_t, v_t):
            """hT_blk(b)->AP node-major block, A_blk(b)->AP; returns nothing (v updated)."""
            pool_ps = ps_sm.tile([128, GCH], F32, tag="pool")
            for bb in range(BLK):
                nc.tensor.matmul(
                    pool_ps[:], hT_blk(bb), A_blk(bb),
                    start=(bb == 0), stop=(bb == BLK - 1),
                )
            pooledT = sm.tile([128, GCH], F16, tag="pooledT")
            if POOLED_DVE:
                nc.vector.tensor_copy(pooledT[:], pool_ps[:])
            else:
                nc.scalar.copy(pooledT[:], pool_ps[:])
            x2_ps = ps_sm.tile([GCH + 1, D], F32, tag="x2", padded_shape=[128, D])
            nc.tensor.matmul(x2_ps[:GCH, :], pooledT[:], Ws_sb[:, l, :], start=True, stop=True)
            nc.tensor.matmul(
                x2_ps[GCH : GCH + 1, :], bcolh_sb[:, l : l + 1], eye_sb[:],
                start=True, stop=True, tile_position=(0, 64),
            )
            x2sb = sm.tile([GCH + 1, D], F16, tag="x2sb")
            nc.scalar.copy(x2sb[:], x2_ps[:])

            xb_ps = ps_big.tile([128, C], F32, tag="xb")
            for ss in range(0, C, 512):
                nc.tensor.matmul(
                    xb_ps[:, ss : ss + 512], W_sb[:, l, :], v_t[:, ss : ss + 512],
                    start=True, stop=False,
                )
            for ss in range(0, C, 512):
                nc.tensor.matmul(
                    xb_ps[:, ss : ss + 512], x2sb[:], AT_t[:, ss : ss + 512],
                    start=False, stop=True,
                )
            # v = min(exp(xb),1) + relu(xb); bias already inside xb via ones row
            e_sb = sm.tile([128, C], F16, tag="e")
            nc.scalar.activation(e_sb[:], xb_ps[:], AF.Exp)
            emin = sm.tile([128, C], F16, tag="emin")
            if GPS_MIN:
                nc.gpsimd.tensor_scalar_min(emin[:], e_sb[:], 1.0)
            else:
                nc.vector.tensor_scalar_min(emin[:], e_sb[:], 1.0)
            nc.vector.scalar_tensor_tensor(v_t[:], xb_ps[:], 0.0, emin[:], OP.max, OP.add)

        def head(k, hT_t):
            pool_ps = ps_sm.tile([128, GCH], F32, tag="pool")
            for bb in range(BLK):
                nc.tensor.matmul(
                    pool_ps[:], hT_t[:, bb, :], hA_sbs[k % M][:, BLK * 128 + bb * GCH : BLK * 128 + (bb + 1) * GCH],
                    start=(bb == 0), stop=(bb == BLK - 1),
                )
            p3 = sm.tile([128, GCH], F32, tag="p3")
            if P3_DVE:
                nc.vector.tensor_copy(p3[:], pool_ps[:])
            else:
                nc.scalar.copy(p3[:], pool_ps[:])
            r1_sbs = []
            r1_ps = ps_sm.tile([128, 2 * GCH], F32, tag="x2")
            for hh in range(2):
                nc.tensor.matmul(
                    r1_ps[:, hh * GCH : (hh + 1) * GCH],
                    wf1_sb[:, hh * 128 : (hh + 1) * 128], p3[:],
                    start=True, stop=True,
                )
                r1_sb = sm.tile([128, GCH], F32, tag=f"r1s_{hh}")
                nc.scalar.activation(
                    r1_sb[:], r1_ps[:, hh * GCH : (hh + 1) * GCH], AF.Relu,
                    bias=bf1_sb[:, hh : hh + 1],
                )
                r1_sbs.append(r1_sb)
            out_ps = ps_sm.tile([T, GCH], F32, tag="pool")
            for hh in range(2):
                nc.tensor.matmul(
                    out_ps[:], wf2_sb[:, hh, :], r1_sbs[hh][:],
                    start=(hh == 0), stop=(hh == 1),
                )
            nc.scalar.activation(
                out_sb[:, k * GCH : (k + 1) * GCH], out_ps[:], AF.Identity, bias=bf2_sb[:]
            )

        hA_sbs = [None] * M
        AT_sbs = [None] * M
        v_ts = [None] * M

        def main():
            for g0 in range(0, nchunk, M):
                ks = list(range(g0, min(g0 + M, nchunk)))
                for k in ks:
                    hA_t = gio.tile([128, BLK * 128 + BLK * GCH], F16, tag="hA", name=f"hA{k}")
                    nc.sync.dma_start(hA_t[:], hA_d[k])
                    AT_t = gio.tile([GCH + 1, C], F16, tag="AT", name=f"AT{k}")
                    nc.sync.dma_start(AT_t[:], AT_d[k])
                    v_t = vp.tile([128, C], F16, tag="v", name=f"v{k}")
                    if VTRANS_DMA:
                        dma_tr(v_t[:], hraw_d[k * C : (k + 1) * C, :])
                    else:
                        pe_transpose(v_t, lambda b, t=hA_t: t[:, b * 128 : (b + 1) * 128])
                    hA_sbs[k % M], AT_sbs[k % M], v_ts[k % M] = hA_t, AT_t, v_t
                for l in range(3):
                    for k in ks:
                        hA_t, AT_t, v_t = hA_sbs[k % M], AT_sbs[k % M], v_ts[k % M]
                        if l == 0:
                            hT_blk = lambda b: hA_t[:, b * 128 : (b + 1) * 128]
                        else:
                            hT_t = hp.tile([128, BLK, 128], F16, tag="hT")
                            if TRANS_DMA:
                                dma_tr(hT_t[:], v_t[:])
                            else:
                                pe_transpose(hT_t, lambda b, t=v_t: t[:, b * 128 : (b + 1) * 128])
                            hT_blk = lambda b, t=hT_t: t[:, b, :]
                        A_blk = lambda b: hA_t[:, BLK * 128 + b * GCH : BLK * 128 + (b + 1) * GCH]
                        pool_x2_xb(l, hT_blk, A_blk, AT_t, v_t)
                for k in ks:
                    v_t = v_ts[k % M]
                    hT_t = hp.tile([128, BLK, 128], F16, tag="hT")
                    if TRANS_DMA:
                        dma_tr(hT_t[:], v_t[:])
                    else:
                        pe_transpose(hT_t, lambda b, t=v_t: t[:, b * 128 : (b + 1) * 128])
                    head(k, hT_t)
            nc.sync.dma_start(out_d, out_sb[:])

        if reps > 1:
            with tc.For_i(0, reps, 1):
                main()
        else:
            main()

    nc._tc_dbg = tc.ordered_instructions_by_block
    nc.compile()
    return nc


# --------------------------------------------------------------------------
# Entry point
# --------------------------------------------------------------------------

_CACHE = {}


def _run(cfg, inputs, reps=1):
    prep = _prepare(
        cfg,
        inputs["h_subgraph"],
        inputs["batch_idx"],
        [inputs["W1"], inputs["W2"], inputs["W3"]],
        [inputs["b1"], inputs["b2"], inputs["b3"]],
        [inputs["Ws1"], inputs["Ws2"], inputs["Ws3"]],
        [inputs["bs1"], inputs["bs2"], inputs["bs3"]],
        inputs["Wf1"],
        inputs["bf1"],
        inputs["Wf2"],
        inputs["bf2"],
    )
    key = (cfg, prep["nchunk"], reps)
    if key not in _CACHE:
        _CACHE[key] = _build(cfg, prep["nchunk"], reps=reps)
    nc = _CACHE[key]
    res = bass_utils.run_bass_kernel_spmd(
        nc, prep["in_maps"], core_ids=list(range(cfg.n_cores))
    )
    out = np.zeros((cfg.G, cfg.T), np.float32)
    for c in range(cfg.n_cores):
        oc = res.results[c]["out"]  # [T, nchunk*GCH]
        pos, gid = prep["asm"][c]
        if len(pos):
            out[gid, :] = oc[:, pos].T
    if prep["empty_row"] is not None:
        out[prep["counts"] == 0, :] = prep["empty_row"]
    return out


def kernel(**inputs):
    return _run(CFG, inputs, reps=1).astype(np.float32)
